# revision 1
# baseline (speedup 1.0000x reference)
"""Trainium2 8-core Bass kernel for a 2-layer-MLP + 2-layer-GCN encoder.

Strategy (graph/data parallel, per the sharding hint):
  - Nodes are assigned to 8 cores x 196 blocks of 64 "slots", degree-balanced
    so every block carries ~the same number of incoming edges.
  - MLP front-end + per-node transforms run feature-major on the node shard.
  - For each GCNConv, the normalized per-node vectors (z * dinv) are
    AllGathered into a bf16 row table in every core's HBM; edges are routed by
    destination core (host-side), grouped by (src-table-chunk, dst-block), and
    the source rows are fetched with SWDGE dma_gather (int16 indices).
  - The D^-1/2 A D^-1/2 segment-sum is a PE matmul per 128-edge tile:
    out[feat, dstslot] += msgs[edge, feat]^T @ S[edge, dstslot], where S is a
    0/1 indicator built on DVE via is_equal(dstoff, iota).  dinv[dst] is
    pulled out of the sum and applied afterwards; biases cancel in
    training-mode BatchNorm.  BN statistics are AllReduced (sum, sumsq).
"""

import math
import os
import numpy as np
import ml_dtypes

import concourse.bacc as bacc
import concourse.bass as bass
import concourse.mybir as mybir
import concourse.tile as tile
from concourse.bass import ap_utils, exact_div, round_up_to_multiple, MemorySpace
from concourse.bass_utils import run_bass_kernel_spmd
from concourse.masks import make_identity

BF16 = mybir.dt.bfloat16
FP32 = mybir.dt.float32
I16 = mybir.dt.int16
NPBF = ml_dtypes.bfloat16
AF = mybir.ActivationFunctionType
ALU = mybir.AluOpType

# ---------------- problem config (full scale; dev overrides via _configure) --
N = 100_000
E = 3_200_000
C = 8
D_IN, H1, H2, HG, Z = 512, 128, 64, 64, 32
EPS_MLP, EPS_GCN = 1e-3, 1e-5
M = 64                       # dst slots per block
CHUNK_VIEW = 32768           # int16-addressable rows of the (2-node) table view
RB = 6                       # blocks per aggregation round
SINGLE_PACKET = False
AGG_REPEAT = 1  # bench amplification: re-run aggregation phases
BENCH_MODE = "full"  # full | nogather | gatheronly
GATHER_QUEUES = False  # spread gathers across 4 SWDGE queues
GATHER_ELEM_X2 = False  # bench: double gc1 gather payload
GATHER_SUB = 0  # >0: split gathers into single_packet sub-calls of this many tiles


def _derived():
    global NPC, NBLK, PADN, TOTV, CHUNK_NODES, NG, NCH
    NPC = N // C
    NBLK = (NPC + M - 1) // M
    PADN = NBLK * M
    TOTV = C * PADN // 2     # table view rows (2 nodes per 256B row)
    CHUNK_NODES = 2 * CHUNK_VIEW
    NCH = (TOTV + CHUNK_VIEW - 1) // CHUNK_VIEW
    NG = 2 * NCH             # groups = chunks x parity


_derived()


def _configure(n, e, chunk_view=32768, rb=6):
    """Dev-scale override (test only)."""
    global N, E, CHUNK_VIEW, RB
    N, E, CHUNK_VIEW, RB = n, e, chunk_view, rb
    _derived()
    _CACHE.clear()


# --------------------------------------------------------------------------
# patched dma_gather: elem_size restriction relaxed to 64B (fw supports any)
def _dma_gather(gp, out_ap, in_ap, idxs_ap, num_idxs, elem_size, elem_step,
                queue_num=0, single_packet=None):
    gp._assert_queue_num(queue_num)
    assert idxs_ap.dtype == mybir.dt.int16
    assert in_ap.dtype == out_ap.dtype
    elem_size_bytes = elem_size * mybir.dt.size(in_ap.dtype)
    assert elem_size_bytes > 0 and elem_size_bytes % 64 == 0
    assert in_ap.space == MemorySpace.DRAM
    assert idxs_ap.space == MemorySpace.SBUF
    assert out_ap.space == MemorySpace.SBUF
    assert ap_utils.ap_is_contiguous(out_ap.ap[1:])
    assert ap_utils.ap_is_contiguous(idxs_ap.ap[1:])
    assert in_ap.ap[-1][1] == elem_size and out_ap.ap[-1][1] == elem_size
    assert out_ap.ap[0][1] * out_ap.ap[1][1] == round_up_to_multiple(num_idxs, 128)
    assert in_ap.ap[0][0] == elem_step
    stride_bytes = elem_step * mybir.dt.size(in_ap.dtype)
    stride_bytes_256 = exact_div(stride_bytes, 256)
    assert stride_bytes_256 < 256
    _in_ap = gp.lower_ap_dma(in_ap, for_custom_bir_dma=True)
    _idxs_ap = gp.lower_ap(idxs_ap)
    _out_ap = gp.lower_ap(out_ap)
    return gp.add_instruction(
        mybir.InstDMAGatherAnt(
            name=gp.bass.get_next_instruction_name(),
            ins=[*_in_ap, _idxs_ap, gp.lower_val_access(gp.to_reg(num_idxs))],
            outs=[_out_ap],
            transpose=False,
            num_idxs=num_idxs,
            elem_size=elem_size,
            stride_bytes_256=stride_bytes_256,
            gen_mode=0,
            single_packet=SINGLE_PACKET if single_packet is None else single_packet,
            queue_num=queue_num,
            sbuf_tokens_per_rank=0,
            sbuf_free_dim_per_rank=0,
            sbuf_free_dim_pad_per_rank=0,
            sbuf_byte_offset=0,
        )
    )


# --------------------------------------------------------------------------
# host-side preparation
def _prep(x, edge_index):
    src = np.asarray(edge_index[0]).astype(np.int64)
    dst = np.asarray(edge_index[1]).astype(np.int64)
    loads = np.bincount(dst, minlength=N).astype(np.int64) + 1
    dinv = (1.0 / np.sqrt(loads.astype(np.float64))).astype(np.float32)

    # --- degree-balanced node -> (core, block, slot) assignment (snake deal)
    NB = C * NBLK
    caps = np.full(NB, M, np.int64)
    caps[NBLK - 1 :: NBLK] = NPC - (NBLK - 1) * M   # last block of each core
    order = np.argsort(-loads, kind="stable")
    binof = np.empty(N, np.int64)
    slotof = np.empty(N, np.int64)
    pos = 0
    j = 0
    fwd = True
    while pos < N:
        open_bins = np.flatnonzero(caps > j)
        if not fwd:
            open_bins = open_bins[::-1]
        take = min(len(open_bins), N - pos)
        nodes = order[pos : pos + take]
        binof[nodes] = open_bins[:take]
        slotof[nodes] = j
        pos += take
        j += 1
        fwd = not fwd
    nid2 = (binof // NBLK) * PADN + (binof % NBLK) * M + slotof
    old_of_new = np.full(C * PADN, -1, np.int64)
    old_of_new[nid2] = np.arange(N)

    # --- edges (incl. self loops), routed by destination
    s_all = np.concatenate([src, np.arange(N, dtype=np.int64)])
    d_all = np.concatenate([dst, np.arange(N, dtype=np.int64)])
    sp = nid2[s_all]
    dp = nid2[d_all]
    core = dp // PADN
    loc = dp % PADN
    HP = PADN // 2
    par = ((sp % PADN) >= HP).astype(np.int64)
    vrow = (sp // PADN) * HP + (sp % PADN) % HP
    g = (vrow // CHUNK_VIEW) * 2 + par
    idxv = (vrow % CHUNK_VIEW).astype(np.int16)
    cell = (g * NBLK + loc // M).astype(np.int64)
    off = (loc % M).astype(np.float32)

    ncell = NG * NBLK
    cnt = np.bincount(core * ncell + cell, minlength=C * ncell).reshape(C, ncell)
    T = (cnt.max(axis=0) + 127) // 128            # tiles per cell (shared)
    cs = np.concatenate([[0], np.cumsum(T)]).astype(np.int64)
    NT = int(cs[-1])
    EPAD = NT * 128

    idx_stream = np.zeros((C, EPAD), np.int16)
    doff_stream = np.full((C, EPAD), -1.0, np.float32)
    key = core * ncell + cell
    oe = np.argsort(key, kind="stable")
    ks = key[oe]
    starts = np.r_[0, np.flatnonzero(np.diff(ks)) + 1]
    runlen = np.diff(np.r_[starts, len(ks)])
    rank = np.arange(len(ks)) - np.repeat(starts, runlen)
    posn = cs[cell[oe]] * 128 + rank
    idx_stream[core[oe], posn] = idxv[oe]
    doff_stream[core[oe], posn] = off[oe]

    # --- per-core input maps
    xb = None
    in_maps = []
    for k in range(C):
        olds = old_of_new[k * PADN : (k + 1) * PADN]
        valid = olds >= 0
        xt = np.zeros((D_IN, PADN), NPBF)
        xt[:, valid] = x[olds[valid]].T.astype(NPBF)
        dl = np.zeros(PADN, np.float32)
        dl[valid] = dinv[olds[valid]]
        dinvr = np.broadcast_to(dl, (M, PADN)).copy()
        ist = idx_stream[k]
        rep = 8 if GATHER_QUEUES else 2
        idxs = np.tile(ist.reshape(-1, 16).T, (rep, 1)).copy()
        doff = doff_stream[k].astype(NPBF).reshape(NT, 128).T.copy()
        in_maps.append({"xt": xt, "dinvr": dinvr, "idxs": idxs, "doff": doff})

    meta = dict(T=tuple(int(t) for t in T), cs=cs, NT=NT, old_of_new=old_of_new)
    return in_maps, meta


# --------------------------------------------------------------------------
# device program
def _bn_coeffs(nc, pool, st, gam, bet, eps, P, n_total, negate=False):
    """From st[P,2] (sum, sumsq) produce scale[P,1], bias[P,1] (fp32)."""
    cnt = [0]
    def t():
        cnt[0] += 1
        nm = f"bnc{P}_{cnt[0]}_{negate}_{eps}"
        return pool.tile([P, 1], FP32, tag=nm, name=nm)
    mean, ex2, var, rec, inv, sc, t1, bi = (t() for _ in range(8))
    nc.vector.tensor_scalar_mul(mean[:], st[:, 0:1], 1.0 / n_total)
    nc.vector.tensor_scalar_mul(ex2[:], st[:, 1:2], 1.0 / n_total)
    nc.vector.tensor_tensor(out=var[:], in0=mean[:], in1=mean[:], op=ALU.mult)
    nc.vector.tensor_tensor(out=var[:], in0=ex2[:], in1=var[:], op=ALU.subtract)
    nc.vector.tensor_scalar_add(var[:], var[:], eps)
    nc.vector.reciprocal(rec[:], var[:])
    nc.scalar.activation(out=inv[:], in_=rec[:], func=AF.Sqrt)
    nc.vector.tensor_tensor(out=sc[:], in0=gam[:], in1=inv[:], op=ALU.mult)
    nc.vector.tensor_tensor(out=t1[:], in0=mean[:], in1=sc[:], op=ALU.mult)
    nc.vector.tensor_tensor(out=bi[:], in0=bet[:], in1=t1[:], op=ALU.subtract)
    if not negate:
        return sc, bi
    nsc, nbi = t(), t()
    nc.vector.tensor_scalar_mul(nsc[:], sc[:], -1.0)
    nc.vector.tensor_scalar_mul(nbi[:], bi[:], -1.0)
    return sc, bi, nsc, nbi


def _allreduce_stats(nc, pool, dram_pool, sums, sqs, P, groups):
    """Reduce partials [P, n] -> AllReduce -> st [P, 2] fp32 tile."""
    arb = pool.tile([P, 2], FP32, tag="arb")
    nc.vector.tensor_reduce(out=arb[:, 0:1], in_=sums[:], axis=mybir.AxisListType.X,
                            op=ALU.add)
    nc.vector.tensor_reduce(out=arb[:, 1:2], in_=sqs[:], axis=mybir.AxisListType.X,
                            op=ALU.add)
    bi = dram_pool.tile([P, 2], FP32, tag="ar_in")
    bo = dram_pool.tile([P, 2], FP32, tag="ar_out")
    nc.sync.dma_start(out=bi[:], in_=arb[:])
    nc.gpsimd.collective_compute("AllReduce", ALU.add, replica_groups=groups,
                                 ins=[bi[:].opt()], outs=[bo[:].opt()])
    st = pool.tile([P, 2], FP32, tag="arb")
    nc.sync.dma_start(out=st[:], in_=bo[:])
    return st


def _build(Tt):
    """Build the 8-core program for tile-count schedule Tt (tuple)."""
    T = np.asarray(Tt, np.int64)
    cs = np.concatenate([[0], np.cumsum(T)]).astype(np.int64)
    NT = int(cs[-1])
    n_rounds = (NBLK + RB - 1) // RB
    groups = [list(range(C))]

    d = max(dd for dd in range(1, 9) if NBLK % dd == 0)
    SUBW = 64 * d
    NSUB = PADN // SUBW
    NCOL = PADN // 128
    ow = 128 * max(dd for dd in range(1, 9) if NCOL % dd == 0)
    NOW = PADN // ow

    nc = bacc.Bacc("TRN2", target_bir_lowering=False, debug=False, num_devices=C,
                   num_swdge_queues=4 if GATHER_QUEUES else 1)

    # ---- I/O
    xt_d = nc.dram_tensor("xt", [D_IN, PADN], BF16, kind="ExternalInput")
    dinvr_d = nc.dram_tensor("dinvr", [M, PADN], FP32, kind="ExternalInput")
    IXP = 128 if GATHER_QUEUES else 32
    idxs_d = nc.dram_tensor("idxs", [IXP, NT * 8], I16, kind="ExternalInput")
    doff_d = nc.dram_tensor("doff", [128, NT], BF16, kind="ExternalInput")
    w1_d = nc.dram_tensor("w1", [D_IN, H1], BF16, kind="ExternalInput")
    w2_d = nc.dram_tensor("w2", [H1, H2], BF16, kind="ExternalInput")
    wg1_d = nc.dram_tensor("wg1", [H2, HG], BF16, kind="ExternalInput")
    wg2_d = nc.dram_tensor("wg2", [HG, Z], BF16, kind="ExternalInput")
    bn_d = {}
    for nm, p in (("g1", H1), ("be1", H1), ("g2", H2), ("be2", H2),
                  ("g3", HG), ("be3", HG), ("g4", Z), ("be4", Z)):
        bn_d[nm] = nc.dram_tensor(nm, [p, 1], FP32, kind="ExternalInput")
    out_d = nc.dram_tensor("out", [PADN, Z], FP32, kind="ExternalOutput")

    # ---- internal DRAM
    tbl1L = nc.dram_tensor("tbl1L", [PADN // 2, 128], BF16)
    tbl2L = nc.dram_tensor("tbl2L", [PADN // 2, 128], BF16)
    tbl1 = nc.dram_tensor("tbl1", [TOTV, 128], BF16, addr_space="Shared")
    tbl2 = nc.dram_tensor("tbl2", [TOTV, 128], BF16, addr_space="Shared")

    with tile.TileContext(nc) as tc:
        with (
            tc.tile_pool(name="const", bufs=1) as cpool,
            tc.tile_pool(name="main", bufs=1) as mpool,
            tc.tile_pool(name="small", bufs=2) as spool,
            tc.tile_pool(name="xp", bufs=1) as xpool,
            tc.tile_pool(name="psum_mm", bufs=1, space="PSUM") as pp_mm,
            tc.tile_pool(name="psum_agg", bufs=RB, space="PSUM") as pp_agg,
            tc.tile_pool(name="psum_tr", bufs=1, space="PSUM") as pp_tr,
            tc.tile_pool(name="dram", bufs=1, space="DRAM") as dpool,
        ):
            # ---- constants
            identb = cpool.tile([128, 128], BF16)
            make_identity(nc, identb[:])
            identf = cpool.tile([128, 128], FP32)
            make_identity(nc, identf[:])
            W = 0
            for r in range(n_rounds):
                b0, b1 = r * RB, min((r + 1) * RB, NBLK)
                for g in range(NG):
                    W = max(W, int(cs[g * NBLK + b1] - cs[g * NBLK + b0]))
            iotaw = cpool.tile([128, M * W], BF16)
            nc.gpsimd.iota(iotaw[:].rearrange("p (i t) -> p i t", t=W),
                           pattern=[[1, M], [0, W]], base=0, channel_multiplier=0,
                           allow_small_or_imprecise_dtypes=True)
            w1 = cpool.tile([128, (D_IN // 128) * H1], BF16, tag="w1")
            nc.sync.dma_start(out=w1[:].rearrange("p (k h) -> p k h", h=H1),
                              in_=w1_d[:, :].rearrange("(k p) h -> p k h", p=128))
            w2 = cpool.tile([H1, H2], BF16)
            nc.sync.dma_start(out=w2[:], in_=w2_d[:])
            wg1 = cpool.tile([H2, HG], BF16)
            nc.sync.dma_start(out=wg1[:], in_=wg1_d[:])
            wg2 = cpool.tile([HG, Z], BF16)
            nc.sync.dma_start(out=wg2[:], in_=wg2_d[:])
            bn = {}
            for nm in bn_d:
                p = bn_d[nm].shape[0]
                bn[nm] = cpool.tile([p, 1], FP32, tag=f"bn_{nm}", name=f"bn_{nm}")
                nc.sync.dma_start(out=bn[nm][:], in_=bn_d[nm][:])

            KC = D_IN // 128

            def mlp_layer(hin, w, fin, fout, tagpre, tagpost, gam, bet):
                """x/h (bf16, [fin<=128... via w tile) -> pre-BN h + stats ->
                ELU-normalized output (chunked scratch)."""
                h = mpool.tile([fout, PADN], BF16, tag=tagpre, name=f"h_{tagpre}")
                p_sum = cpool.tile([fout, NSUB], FP32, tag=f"ps_{tagpre}",
                                   name=f"ps_{tagpre}")
                p_sq = cpool.tile([fout, NSUB], FP32, tag=f"pq_{tagpre}",
                                  name=f"pq_{tagpre}")
                nhalf = 4 if NSUB % 4 == 0 else (2 if NSUB % 2 == 0 else 1)
                hw_ = PADN // nhalf
                subs_per_half = NSUB // nhalf
                for si in range(NSUB):
                    n0 = si * SUBW
                    if hin is None and si % subs_per_half == 0:
                        half = si // subs_per_half
                        xks = []
                        for kc in range(KC):
                            xk = xpool.tile([128, hw_], BF16, tag=f"xt{kc}",
                                            name=f"xt{kc}")
                            nc.sync.dma_start(
                                out=xk[:],
                                in_=xt_d[kc * 128 : (kc + 1) * 128,
                                         half * hw_ : (half + 1) * hw_])
                            xks.append(xk)
                    ps = pp_mm.tile([128, SUBW], FP32, tag="mm")
                    if hin is None:
                        nh = n0 - (si // subs_per_half) * hw_
                        for kc in range(KC):
                            nc.tensor.matmul(out=ps[:],
                                             lhsT=w[:, kc * H1 : (kc + 1) * H1],
                                             rhs=xks[kc][:, nh : nh + SUBW],
                                             start=(kc == 0),
                                             stop=(kc == KC - 1))
                    else:
                        nc.tensor.matmul(out=ps[:fout, :], lhsT=w[:],
                                         rhs=hin[:, n0 : n0 + SUBW],
                                         start=True, stop=True)
                    nc.scalar.activation(out=h[:, n0 : n0 + SUBW], in_=ps[:fout, :],
                                         func=AF.Copy,
                                         accum_out=p_sum[:, si : si + 1])
                    sq = spool.tile([fout, SUBW], BF16, tag="sq")
                    nc.scalar.activation(out=sq[:], in_=ps[:fout, :], func=AF.Square,
                                         accum_out=p_sq[:, si : si + 1])
                st = _allreduce_stats(nc, cpool, dpool, p_sum, p_sq, fout, groups)
                sc, bi, nsc, nbi = _bn_coeffs(nc, cpool, st, gam, bet, EPS_MLP,
                                              fout, N, negate=True)
                hn = mpool.tile([fout, PADN], BF16, tag=tagpost,
                                name=f"hn_{tagpost}")
                for si in range(NSUB):
                    n0 = si * SUBW
                    hs = (slice(None), slice(n0, n0 + SUBW))
                    scr = spool.tile([fout, SUBW], BF16, tag="scr")
                    nc.scalar.activation(out=hn[hs], in_=h[hs], func=AF.Relu,
                                         bias=bi[:], scale=sc[:])
                    nc.scalar.activation(out=scr[:], in_=h[hs], func=AF.Relu,
                                         bias=nbi[:], scale=nsc[:])
                    nc.scalar.activation(out=scr[:], in_=scr[:], func=AF.Exp,
                                         scale=-1.0)
                    nc.vector.tensor_tensor(out=hn[hs], in0=hn[hs], in1=scr[:],
                                            op=ALU.add)
                    nc.vector.tensor_scalar_add(hn[hs], hn[hs], -1.0)
                if NPC < PADN:
                    nc.vector.memset(hn[:, NPC:], 0.0)
                return hn

            # ---- helper: z = W^T h, scaled by dinv -> 2-half table + AllGather
            def make_table(hsrc, w, fout, tblL, tbl_sh):
                zd = mpool.tile([fout, PADN], BF16, tag="P2", name=f"zd{fout}")
                for si in range(NSUB):
                    n0 = si * SUBW
                    ps = pp_mm.tile([128, SUBW], FP32, tag="mm")
                    nc.tensor.matmul(out=ps[:fout, :], lhsT=w[:],
                                     rhs=hsrc[:, n0 : n0 + SUBW], start=True,
                                     stop=True)
                    dv = spool.tile([fout, SUBW], FP32, tag="dv")
                    nc.sync.dma_start(out=dv[:], in_=dinvr_d[:fout, n0 : n0 + SUBW])
                    nc.vector.tensor_tensor(out=zd[:, n0 : n0 + SUBW],
                                            in0=ps[:fout, :], in1=dv[:], op=ALU.mult)
                zt = mpool.tile([128, NCOL * fout], BF16, tag="P4",
                                name=f"zt{fout}")
                for c in range(NCOL):
                    pt = pp_tr.tile([128, M], BF16, tag="tr")
                    nc.tensor.transpose(out=pt[:, :fout],
                                        in_=zd[:, c * 128 : (c + 1) * 128],
                                        identity=identb[:fout, :fout])
                    nc.scalar.activation(out=zt[:, c * fout : (c + 1) * fout],
                                         in_=pt[:, :fout], func=AF.Copy)
                HC = NCOL // 2  # node chunks per half
                for half in (0, 1):
                    nc.sync.dma_start(
                        out=tblL[:, half * fout : (half + 1) * fout].rearrange(
                            "(c p) f -> p c f", p=128),
                        in_=zt[:, half * HC * fout : (half + 1) * HC * fout].rearrange(
                            "p (c f) -> p c f", f=fout),
                    )
                nc.gpsimd.collective_compute("AllGather", ALU.bypass,
                                             replica_groups=groups,
                                             ins=[tblL[:].opt()],
                                             outs=[tbl_sh[:].opt()])

            # ---- helper: aggregation (+ dinv scale per block, round stats)
            def aggregate(tbl_sh, fout, tag, out_dt):
                hg = mpool.tile([fout, PADN], out_dt, tag="P1",
                                name=f"hg{tag}")
                p_sum = cpool.tile([fout, n_rounds], FP32, tag=f"gps{tag}",
                                   name=f"gps{tag}")
                p_sq = cpool.tile([fout, n_rounds], FP32, tag=f"gpq{tag}",
                                  name=f"gpq{tag}")
                for r in range(n_rounds):
                    b0, b1 = r * RB, min((r + 1) * RB, NBLK)
                    span = (b1 - b0) * M
                    dvr = spool.tile([fout, RB * M], FP32, tag="dvr")
                    nc.sync.dma_start(out=dvr[:, :span],
                                      in_=dinvr_d[:fout, b0 * M : b1 * M])
                    pst = {}
                    for b in range(b0, b1):
                        pst[b] = pp_agg.tile([M, M], FP32, tag="agg",
                                             name=f"agg{tag}_{b}")
                    done = {b: 0 for b in range(b0, b1)}
                    tot = {b: int(sum(T[g * NBLK + b] for g in range(NG)))
                           for b in range(b0, b1)}
                    for g in range(NG):
                        ts = int(cs[g * NBLK + b0])
                        te = int(cs[g * NBLK + b1])
                        nt = te - ts
                        if nt == 0:
                            continue
                        ix = spool.tile([IXP, nt * 8], I16, tag="ix")
                        nc.sync.dma_start(out=ix[:], in_=idxs_d[:, ts * 8 : te * 8])
                        do = spool.tile([128, nt], BF16, tag="do")
                        nc.sync.dma_start(out=do[:], in_=doff_d[:, ts:te])
                        s_t = spool.tile([128, M * nt], BF16, tag="S")
                        nc.vector.tensor_tensor(
                            out=s_t[:].rearrange("p (i t) -> p i t", t=nt),
                            in0=do[:].unsqueeze(1).broadcast_to([128, M, nt]),
                            in1=iotaw[:].rearrange("p (i t) -> p i t", t=W)[:, :, :nt],
                            op=ALU.is_equal)
                        fs = 128 if (GATHER_ELEM_X2 and fout == 64) else fout
                        m_t = spool.tile([128, nt * fs], BF16,
                                         tag="m" if fs == fout else "mx2",
                                         name="m_t")
                        ch, par = g >> 1, g & 1
                        rb_ = ch * CHUNK_VIEW
                        rows = min(CHUNK_VIEW, TOTV - rb_)
                        if BENCH_MODE != "nogather":
                            es = fs
                            cl = 0 if fs != fout else par * fout
                            inap = tbl_sh[rb_ : rb_ + rows, cl : cl + es]
                            if GATHER_SUB:
                                mv = m_t[:].rearrange("p (t f) -> p t f", f=es)
                                for t0 in range(0, nt, GATHER_SUB):
                                    t1 = min(t0 + GATHER_SUB, nt)
                                    _dma_gather(
                                        nc.gpsimd,
                                        out_ap=mv[:, t0:t1, :],
                                        in_ap=inap,
                                        idxs_ap=ix[:, t0 * 8 : t1 * 8],
                                        num_idxs=(t1 - t0) * 128,
                                        elem_size=es,
                                        elem_step=128,
                                        single_packet=True,
                                        queue_num=(g % 4) if GATHER_QUEUES else 0,
                                    )
                            else:
                                _dma_gather(
                                    nc.gpsimd,
                                    out_ap=mv if False else m_t[:].rearrange(
                                        "p (t f) -> p t f", f=es),
                                    in_ap=inap,
                                    idxs_ap=ix[:],
                                    num_idxs=nt * 128,
                                    elem_size=es,
                                    elem_step=128,
                                    queue_num=(g % 4) if GATHER_QUEUES else 0,
                                )
                        else:
                            nc.vector.memset(m_t[:, 0:fout], 0.0)
                        gonly = BENCH_MODE == "gatheronly"
                        for b in range(b0, b1):
                            tb = int(cs[g * NBLK + b] - ts)
                            for t in range(int(T[g * NBLK + b])):
                                if gonly and done[b] not in (0, tot[b] - 1):
                                    done[b] += 1
                                    continue
                                nc.tensor.matmul(
                                    out=pst[b][:fout, :],
                                    lhsT=m_t[:, (tb + t) * fs : (tb + t) * fs + fout],
                                    rhs=s_t[:].rearrange("p (i t) -> p i t",
                                                         t=nt)[:, :, tb + t],
                                    start=(done[b] == 0),
                                    stop=(done[b] == tot[b] - 1))
                                done[b] += 1
                    for b in range(b0, b1):
                        if tot[b] == 0:
                            continue
                        nc.vector.tensor_tensor(
                            out=hg[:, b * M : (b + 1) * M], in0=pst[b][:fout, :],
                            in1=dvr[:, (b - b0) * M : (b - b0 + 1) * M],
                            op=ALU.mult)
                    nc.vector.tensor_reduce(out=p_sum[:, r : r + 1],
                                            in_=hg[:, b0 * M : b1 * M],
                                            axis=mybir.AxisListType.X, op=ALU.add)
                    sq = spool.tile([fout, RB * M], BF16, tag="gsq")
                    nc.scalar.activation(out=sq[:, :span],
                                         in_=hg[:, b0 * M : b1 * M],
                                         func=AF.Square,
                                         accum_out=p_sq[:, r : r + 1])
                st = _allreduce_stats(nc, cpool, dpool, p_sum, p_sq, fout, groups)
                return hg, st

            # ============ phases
            h1n = mlp_layer(None, w1, D_IN, H1, "P1", "P2", bn["g1"], bn["be1"])
            h2n = mlp_layer(h1n, w2, H1, H2, "P1", "P3", bn["g2"], bn["be2"])
            make_table(h2n, wg1, HG, tbl1L, tbl1)
            for _rep in range(AGG_REPEAT - 1):
                aggregate(tbl1, HG, f"1r{_rep}", BF16)
            hg1, st3 = aggregate(tbl1, HG, "1", BF16)
            sc3, bi3 = _bn_coeffs(nc, cpool, st3, bn["g3"], bn["be3"], EPS_GCN,
                                  HG, N)
            h3 = mpool.tile([HG, PADN], BF16, tag="P3", name="h3")
            nc.scalar.activation(out=h3[:], in_=hg1[:], func=AF.Relu, bias=bi3[:],
                                 scale=sc3[:])
            make_table(h3, wg2, Z, tbl2L, tbl2)
            for _rep in range(AGG_REPEAT - 1):
                aggregate(tbl2, Z, f"2r{_rep}", FP32)
            hg2, st4 = aggregate(tbl2, Z, "2", FP32)
            sc4, bi4 = _bn_coeffs(nc, cpool, st4, bn["g4"], bn["be4"], EPS_GCN,
                                  Z, N)
            ot = mpool.tile([128, NCOL * Z], FP32, tag="P4", name="ot")
            for so in range(NOW):
                o0 = so * ow
                oc = spool.tile([Z, ow], FP32, tag="oc")
                nc.vector.tensor_scalar(out=oc[:], in0=hg2[:, o0 : o0 + ow],
                                        scalar1=sc4[:], scalar2=bi4[:],
                                        op0=ALU.mult, op1=ALU.add)
                for cc in range(ow // 128):
                    c = so * (ow // 128) + cc
                    pt = pp_tr.tile([128, M], FP32, tag="tr")
                    nc.tensor.transpose(out=pt[:, :Z],
                                        in_=oc[:, cc * 128 : (cc + 1) * 128],
                                        identity=identf[:Z, :Z])
                    nc.vector.tensor_copy(out=ot[:, c * Z : (c + 1) * Z],
                                          in_=pt[:, :Z])
            nc.sync.dma_start(
                out=out_d[:, :].rearrange("(c p) f -> p c f", p=128),
                in_=ot[:].rearrange("p (c f) -> p c f", f=Z),
            )

    nc.compile()
    return nc


# --------------------------------------------------------------------------
_CACHE = {}


def _get_program(Tt):
    key = (Tt, AGG_REPEAT, BENCH_MODE, GATHER_QUEUES, GATHER_ELEM_X2, GATHER_SUB)
    if key not in _CACHE:
        _CACHE[key] = _build(Tt)
    return _CACHE[key]


def _full_in_maps(in_maps, W1, W2, Wg1, Wg2, g1, be1, g2, be2, g3, be3, g4, be4):
    shared = {
        "w1": np.asarray(W1).astype(NPBF),
        "w2": np.asarray(W2).astype(NPBF),
        "wg1": np.asarray(Wg1).astype(NPBF),
        "wg2": np.asarray(Wg2).astype(NPBF),
        "g1": np.asarray(g1, np.float32).reshape(-1, 1),
        "be1": np.asarray(be1, np.float32).reshape(-1, 1),
        "g2": np.asarray(g2, np.float32).reshape(-1, 1),
        "be2": np.asarray(be2, np.float32).reshape(-1, 1),
        "g3": np.asarray(g3, np.float32).reshape(-1, 1),
        "be3": np.asarray(be3, np.float32).reshape(-1, 1),
        "g4": np.asarray(g4, np.float32).reshape(-1, 1),
        "be4": np.asarray(be4, np.float32).reshape(-1, 1),
    }
    return [dict(m, **shared) for m in in_maps]


def kernel(x, edge_index, W1, b1, g1, be1, W2, b2, g2, be2,
           Wg1, bg1, g3, be3, Wg2, bg2, g4, be4):
    # b1/b2/bg1/bg2 cancel exactly in training-mode BatchNorm.
    x = np.asarray(x)
    in_maps, meta = _prep(x, edge_index)
    maps = _full_in_maps(in_maps, W1, W2, Wg1, Wg2, g1, be1, g2, be2, g3, be3,
                         g4, be4)
    nc = _get_program(meta["T"])
    res = run_bass_kernel_spmd(nc, maps, core_ids=list(range(C)))
    out = np.empty((N, Z), np.float32)
    oon = meta["old_of_new"]
    for k in range(C):
        olds = oon[k * PADN : (k + 1) * PADN]
        valid = olds >= 0
        out[olds[valid]] = res.results[k]["out"][valid]
    return out

